# revision 1
# baseline (speedup 1.0000x reference)
"""Trainium2 Bass kernel for: softmax2d(channel) -> channel mix -> bias ->
RReLU(0.2 eval) -> relu(act + 0.1*x).

Full-input contract: kernel(**inputs) takes the complete tensors and returns
the complete output. Internally shards batch B=16 across 8 NeuronCores
(2 batches/core). Per-core layout: [128 partitions = 2 batches x 64 channels,
65536 free = H*W].

Math restructure: with S[b,n] = sum_c e[(b,c),n], rb = 1/S, and
W'[(b,c),(b,d)] = mix[d,c] + bias[d] (bias folded into the weights),
  V' = W' @ e            = S * (mix @ softmax + bias)   (unnormalized)
  prelu(rb * V') = rb * prelu(V')                        (rb > 0)
so the two PE matmuls (V' and the block-ones sum SB) both depend only on e
and run in parallel; normalization happens once, post-activation.

Pipeline per [128, TILE_N] tile:
  e  = exp(x)                 ACT (Exp) -> float32r
  SB = blockones @ e          PE -> PSUM     | V' = W' @ e   PE -> PSUM
  rb = recip(SB)              DVE approx     | aa = prelu(V') ACT
  t  = aa * rb                GpSimd tensor_tensor
  z  = 0.1*x + t              DVE scalar_tensor_tensor
  out= max(z, 0)              DVE tensor_scalar (in place)
"""

import numpy as np

B, C, H, W = 16, 64, 256, 256
N_CORES = 8
BPC = B // N_CORES          # batches per core
P = BPC * C                 # 128 partitions
F = H * W                   # 65536 free columns per core
TILE_N = 2048               # ACT/DVE SBUF tile width
PS_N = 1024                 # PSUM tile width (2 banks of f32)
MM_N = 512                  # single matmul max free dim (one PSUM bank, f32)
RRELU_SLOPE = 0.2
X_BUFS = 7                  # input prefetch depth
Z_BUFS = 3
MID_BUFS = 3
E_BUFS = 3
OUT_N = 1024
OUT_DMA = "scalar"
POOL_FRAC_NUM = 7
POOL_FRAC_DEN = 8
PS_BUFS = 2
SKEWS = (0, 1, 2, 3, 4, 5)  # emission offsets: load, exp, mm, act, mult, out

# "float32" (exact, PE 4 cyc/row) or "float32r" (PE 1 cyc/row, ~1e-4 rel err)
MM_DTYPE = "float32r"

_CACHE = {}

RES_RELU_NAME = "RESIDUAL_RELU_NN11888"


def _residual_relu_op():
    """Fused DVE op: out = relu(s0*in0 + in1). Registered at runtime via
    the dve_ops extension registry (3 ALU slices)."""
    import numpy as np_
    import concourse.dve_ops as dve_ops
    from concourse.dve_spec import Spec, Src0, Src1, C0, relu, lower, _has_src1
    from concourse.dve_uop import DveOpSpec

    for op in dve_ops.OPS:
        if op.name == RES_RELU_NAME:
            return op
    spec = Spec(
        body=relu(Src0 * C0 + Src1),
        reference=lambda in0, in1, s0, s1, imm2: np_.maximum(in0 * s0 + in1, 0),
    )
    op = dve_ops.DveOp(RES_RELU_NAME, spec, subdim=False, uops_sha={})
    row = max(dve_ops._SUB_OPCODE_FOR_NAME.values()) + 1
    assert row < 0x20
    dve_ops.OPS.append(op)
    dve_ops._SUB_OPCODE_FOR_NAME[RES_RELU_NAME] = row
    dve_ops.CUSTOM_DVE_SPECS[RES_RELU_NAME] = spec
    for ver in ("v3", "v4"):
        dve_ops._COMPILE_CACHE[(RES_RELU_NAME, ver)] = DveOpSpec(
            name=RES_RELU_NAME,
            opcode=row,
            uops=lower(spec, ver=ver),
            rd1_en=_has_src1(spec),
        )
    return op


def _build_nc():
    import concourse.bacc as bacc
    import concourse.mybir as mybir
    import concourse.tile as tile

    f32 = mybir.dt.float32
    AF = mybir.ActivationFunctionType
    OP = mybir.AluOpType
    mm_dt = getattr(mybir.dt, MM_DTYPE)

    nc = bacc.Bacc(
        "TRN2",
        target_bir_lowering=False,
        debug=False,
        enable_asserts=False,
    )

    x_d = nc.dram_tensor("x", [P, F], f32, kind="ExternalInput").ap()
    wblk_d = nc.dram_tensor("wblk", [P, P], f32, kind="ExternalInput").ap()
    ones_d = nc.dram_tensor("onesblk", [P, P], f32, kind="ExternalInput").ap()
    out_d = nc.dram_tensor("out", [P, F], f32, kind="ExternalOutput").ap()

    OUT_DMA_ENGINE = getattr(nc, OUT_DMA)

    with tile.TileContext(nc) as tc:
        with (
            tc.tile_pool(name="const", bufs=1) as const,
            tc.tile_pool(name="io", bufs=3) as io,
            tc.tile_pool(name="mid", bufs=MID_BUFS) as mid,
            tc.tile_pool(name="ps_sb", bufs=PS_BUFS, space="PSUM") as ps_sb,
            tc.tile_pool(name="ps_u", bufs=PS_BUFS, space="PSUM") as ps_u,
        ):
            w_mix = const.tile([P, P], mm_dt)
            nc.sync.dma_start(out=w_mix[:], in_=wblk_d[:].bitcast(mm_dt))
            onesblk = const.tile([P, P], mm_dt)
            nc.sync.dma_start(out=onesblk[:], in_=ones_d[:].bitcast(mm_dt))

            ntiles = F // TILE_N
            st = {}  # per-tile live state

            def stage_load(ti):
                x_t = io.tile(
                    [P, TILE_N], f32, bufs=X_BUFS, name=f"x_{ti}", tag="x_t"
                )
                nc.sync.dma_start(
                    out=x_t[:], in_=x_d[:, ti * TILE_N : (ti + 1) * TILE_N]
                )
                st[ti] = {"x": x_t}

            def stage_exp(ti):
                e_t = mid.tile(
                    [P, TILE_N], mm_dt, name=f"e_{ti}", tag="e_t", bufs=E_BUFS
                )
                nc.scalar.activation(e_t[:], st[ti]["x"][:], AF.Exp)
                st[ti]["e"] = e_t

            def stage_mm(ti):
                e_t = st[ti]["e"]
                chunks = []
                for kp in range(0, TILE_N, PS_N):
                    sb_c = ps_sb.tile([P, PS_N], f32, tag="sb_c")
                    u_c = ps_u.tile([P, PS_N], f32, tag="u_c")
                    for k in range(kp, kp + PS_N, MM_N):
                        nc.tensor.matmul(
                            sb_c[:, k - kp : k - kp + MM_N],
                            onesblk[:],
                            e_t[:, k : k + MM_N],
                            start=True,
                            stop=True,
                        )
                    for k in range(kp, kp + PS_N, MM_N):
                        nc.tensor.matmul(
                            u_c[:, k - kp : k - kp + MM_N],
                            w_mix[:],
                            e_t[:, k : k + MM_N],
                            start=True,
                            stop=True,
                        )
                    chunks.append((kp, sb_c, u_c))
                st[ti]["chunks"] = chunks

            def stage_act(ti):
                rb_t = mid.tile([P, TILE_N], f32, name=f"rb_{ti}", tag="rb_t")
                aa_t = mid.tile([P, TILE_N], f32, name=f"aa_{ti}", tag="aa_t")
                for kp, sb_c, u_c in st[ti].pop("chunks"):
                    psl = slice(kp, kp + PS_N)
                    nc.vector.reciprocal_approx_fast(
                        out=rb_t[:, psl], in_=sb_c[:]
                    )
                    nc.scalar.activation(
                        aa_t[:, psl],
                        u_c[:],
                        AF.Prelu,
                        bias=0.0,
                        scale=1.0,
                        alpha=RRELU_SLOPE,
                    )
                st[ti]["rb"] = rb_t
                st[ti]["aa"] = aa_t

            def stage_mult(ti):
                t_t = mid.tile([P, TILE_N], f32, name=f"t_{ti}", tag="t_t")
                h = (TILE_N * POOL_FRAC_NUM) // POOL_FRAC_DEN
                nc.gpsimd.tensor_tensor(
                    t_t[:, :h], st[ti]["aa"][:, :h], st[ti]["rb"][:, :h], OP.mult
                )
                nc.vector.tensor_tensor(
                    t_t[:, h:], st[ti]["aa"][:, h:], st[ti]["rb"][:, h:], OP.mult
                )
                st[ti]["t"] = t_t

            res_relu = _residual_relu_op()

            def stage_out(ti):
                z_t = io.tile(
                    [P, TILE_N], f32, bufs=Z_BUFS, name=f"z_{ti}", tag="z_t"
                )
                for ko in range(0, TILE_N, OUT_N):
                    osl = slice(ko, ko + OUT_N)
                    nc.vector._custom_dve(
                        res_relu,
                        out=z_t[:, osl],
                        in0=st[ti]["x"][:, osl],
                        in1=st[ti]["t"][:, osl],
                        s0=0.1,
                    )
                    OUT_DMA_ENGINE.dma_start(
                        out=out_d[:, ti * TILE_N + ko : ti * TILE_N + ko + OUT_N],
                        in_=z_t[:, osl],
                    )
                del st[ti]

            stages = [stage_load, stage_exp, stage_mm, stage_act, stage_mult, stage_out]
            offs = SKEWS
            assert len(offs) == len(stages)
            maxoff = offs[-1]
            for step in range(ntiles + maxoff):
                for si in reversed(range(len(stages))):
                    ti = step - offs[si]
                    if 0 <= ti < ntiles:
                        stages[si](ti)

    nc.compile()
    return nc


def _get_nc():
    if "nc" not in _CACHE:
        _CACHE["nc"] = _build_nc()
    return _CACHE["nc"]


def _make_in_maps(x, mix, bias):
    x = np.ascontiguousarray(np.asarray(x, dtype=np.float32))
    mix = np.asarray(mix, dtype=np.float32)
    bias = np.asarray(bias, dtype=np.float32)

    xs = x.reshape(N_CORES, P, F)

    # lhsT layout: V'[(b,d),n] = sum_{(b',c)} wblk[(b',c),(b,d)] * e[(b',c),n]
    # wblk[(b,c),(b,d)] = mix[d,c] + bias[d]  (bias folded: sums to bias*S)
    blk = (mix.T + bias[None, :]).astype(np.float32)
    wblk = np.zeros((P, P), np.float32)
    wblk[0:C, 0:C] = blk
    wblk[C : 2 * C, C : 2 * C] = blk

    onesblk = np.zeros((P, P), np.float32)
    onesblk[0:C, 0:C] = 1.0
    onesblk[C : 2 * C, C : 2 * C] = 1.0

    return [
        {"x": xs[c], "wblk": wblk, "onesblk": onesblk}
        for c in range(N_CORES)
    ]


def run(inputs, trace=False):
    from concourse.bass_utils import run_bass_kernel_spmd

    nc = _get_nc()
    in_maps = _make_in_maps(inputs["x"], inputs["mix"], inputs["bias"])
    res = run_bass_kernel_spmd(nc, in_maps, list(range(N_CORES)), trace=trace)
    out = np.stack([res.results[c]["out"] for c in range(N_CORES)])
    return out.reshape(B, C, H, W), res


def kernel(x, mix, bias):
    out, _ = run({"x": x, "mix": mix, "bias": bias})
    return out.astype(np.float32, copy=False)

